# revision 8
# baseline (speedup 1.0000x reference)
"""Block-diagonal linear (grouped GEMM) on 8 TRN2 NeuronCores — v6.

out[b, g*512+n] = sum_k x[b, g*512+k] * blocks[g, k, n]

Group-parallel: core g computes block g's GEMM in bf16 (fp32 PSUM).

Key idea vs earlier versions: the head was INPUT-SUPPLY bound, and the
supply ceiling (~200GB/s) was the DMA queue's in-flight cap (~4
transfers) x round-trip latency with only 128KB per transfer (128
descriptors x 1KB partition rows). The host already repacks x, so it
now writes x (and reads the output) in SBUF-layout order:

    xT_shuf[p, KT*m0 + j*c + t] = x[m0+t, g*512 + j*128 + p]

per token-chunk [m0, m0+c). Each chunk then loads as ONE 2D DMA of
128 descriptors x (4c x 2B) contiguous bytes — 0.5-1MB per in-flight
slot — and each chunk's output stores the same way. Both HWDGE rings
stay far ahead of the PE with ~28 total DMAs; the SWDGE ring is not
used at all. The final 32-token chunk packs all 4 nt-groups in one
PSUM bank whose flat layout IS the packed output layout: one cast +
one 32KB DMA after the last matmul.

Other trace-driven details:
 - W rides first on the sync ring (j0 slice ahead of j1-3) while
   chunk0's x halves ride the scalar ring, so matmul j0 starts as
   soon as the first two transfers land (~9.5us).
 - warmup dummies fill the issue->data window so the HAM clock-gate
   lifts right as real work starts.
 - all PSUM->SBUF casts on DVE; outp bufs=3 so a tail cast never
   waits on a previous chunk's output DMA.
 - exec window = first useful inst .. last inst; the ~8.7us NEFF
   teardown after the last DMA is fixed cost.
"""
import numpy as np
import ml_dtypes

import concourse.bacc as bacc
import concourse.tile as tile
from concourse import mybir
from concourse.bass_utils import run_bass_kernel_spmd

TOKENS = 8192
G = 8
M = 512
N = 512
P = 128
KT = M // P
NT = N // P
SUB = 512
F32 = mybir.dt.float32
BF16 = mybir.dt.bfloat16
NPBF16 = ml_dtypes.bfloat16

CHUNKS = [512, 512, 1024, 1024, 1024, 1024, 1024, 1024, 512, 256, 128, 96, 32]
assert sum(CHUNKS) == TOKENS
N_WARM = 20

_CACHE: dict = {}


def _body(tc, nc, xT, w, outT):
    with (
        tc.tile_pool(name="wp", bufs=1) as wp,
        tc.tile_pool(name="xin", bufs=8) as xin,
        tc.tile_pool(name="outp", bufs=3) as outp,
        tc.tile_pool(name="pso", bufs=8, space="PSUM") as pso,
    ):
        w_r = wp.tile([P, KT, N], BF16, tag="wr")

        # Head: W j0 slice first on sync; chunk0's two x halves on
        # scalar — matmul j0 needs only the first transfer of each ring.
        c0 = CHUNKS[0]
        x_t0 = xin.tile([P, KT * c0], BF16, tag="x", name="x_t0")
        nc.sync.dma_start(w_r[:, 0, :], w[:, 0:N])
        nc.scalar.dma_start(x_t0[:, :c0], xT[:, 0:c0])
        nc.sync.dma_start(w_r[:, 1:4, :], w[:, N:KT * N])
        nc.scalar.dma_start(x_t0[:, c0:KT * c0], xT[:, c0:KT * c0])

        # HAM warm-up: dependency-free dummy matmuls over zeroed SBUF
        # into a scratch PSUM bank while the first DMAs land.
        warm_x = xin.tile([P, 2 * P], BF16, tag="wx")
        warm_ps = pso.tile([P, SUB], F32, tag="pso")
        nc.vector.memset(warm_x, 0)
        for _ in range(N_WARM):
            nc.tensor.matmul(
                warm_ps[:, :P], warm_x[:, :P], warm_x[:, P:2 * P],
                start=True, stop=True,
            )
        # throwaway ACT copy: pulls the one-time ~1.3us ACT_TABLE_LOAD
        # into the warm-up shadow instead of the tail casts below
        nc.scalar.copy(warm_x[:, P:2 * P], warm_ps[:, :P])

        m0 = 0
        for ci, c in enumerate(CHUNKS):
            if ci == 0:
                x_t = x_t0
            else:
                x_t = xin.tile([P, KT * c], BF16, tag="x")
                eng = nc.sync if ci % 2 == 1 else nc.scalar
                eng.dma_start(x_t, xT[:, KT * m0:KT * (m0 + c)])
            xv = x_t.rearrange("p (j t) -> p j t", j=KT)
            oeng = nc.scalar if ci % 2 == 1 else nc.sync

            if ci == len(CHUNKS) - 1:
                # final 32-token chunk: 4 nt-groups in ONE PSUM bank;
                # the bank's flat [nt, t] layout IS the packed output
                # layout -> one cast + one small DMA
                ps_o = pso.tile([P, SUB], F32, tag="pso")
                for nt in range(NT):
                    for j in range(KT):
                        nc.tensor.matmul(
                            ps_o[:, nt * c:(nt + 1) * c],
                            w_r[:, j, nt * P:(nt + 1) * P],
                            xv[:, j, :],
                            start=(j == 0),
                            stop=(j == KT - 1),
                        )
                otl = outp.tile([P, NT * c], BF16, tag="o", name="otl")
                nc.vector.tensor_copy(otl, ps_o[:, :NT * c])
                nc.sync.dma_start(outT[:, NT * m0:NT * (m0 + c)], otl)
                m0 += c
                continue

            ot = outp.tile([P, NT * c], BF16, tag="o")
            ov = ot.rearrange("p (nt t) -> p nt t", nt=NT)
            if ci == 0:
                # j-outer: the first 4 matmuls need only w_j0 + x_j01
                pss0 = [
                    pso.tile([P, SUB], F32, tag="pso", name=f"ps0_{nt}")
                    for nt in range(NT)
                ]
                for j in range(KT):
                    for nt in range(NT):
                        nc.tensor.matmul(
                            pss0[nt][:, :c],
                            w_r[:, j, nt * P:(nt + 1) * P],
                            xv[:, j, :],
                            start=(j == 0),
                            stop=(j == KT - 1),
                        )
                for nt in range(NT):
                    nc.vector.tensor_copy(ov[:, nt, :], pss0[nt][:, :c])
            else:
                for s0 in range(0, c, SUB):
                    sw = min(SUB, c - s0)
                    for nt in range(NT):
                        ps_o = pso.tile([P, SUB], F32, tag="pso")
                        for j in range(KT):
                            nc.tensor.matmul(
                                ps_o[:, :sw],
                                w_r[:, j, nt * P:(nt + 1) * P],
                                xv[:, j, s0:s0 + sw],
                                start=(j == 0),
                                stop=(j == KT - 1),
                            )
                        # tail chunks: split casts over DVE and ACT so
                        # the drain after the last matmuls stays short
                        if ci >= len(CHUNKS) - 4 and nt % 2 == 1:
                            nc.scalar.copy(ov[:, nt, s0:s0 + sw], ps_o[:, :sw])
                        else:
                            nc.vector.tensor_copy(ov[:, nt, s0:s0 + sw], ps_o[:, :sw])
            oeng.dma_start(outT[:, NT * m0:NT * (m0 + c)], ot)
            m0 += c


def _build():
    nc = bacc.Bacc("TRN2", target_bir_lowering=False, debug=False, num_devices=G)
    xT = nc.dram_tensor("xT", [P, KT * TOKENS], BF16, kind="ExternalInput").ap()
    w = nc.dram_tensor("w", [P, KT * N], BF16, kind="ExternalInput").ap()
    outT = nc.dram_tensor("outT", [P, NT * TOKENS], BF16, kind="ExternalOutput").ap()
    with tile.TileContext(nc) as tc:
        _body(tc, nc, xT, w, outT)
    nc.compile()
    return nc


def _run(in_maps, **kwargs):
    if "nc" not in _CACHE:
        _CACHE["nc"] = _build()
    return run_bass_kernel_spmd(_CACHE["nc"], in_maps, list(range(G)), **kwargs)


def _pack_x(xg):
    """[TOKENS, 512] -> [128, KT*TOKENS] chunk-packed SBUF layout."""
    outp = np.empty((P, KT * TOKENS), dtype=NPBF16)
    m0 = 0
    for c in CHUNKS:
        blk = xg[m0:m0 + c, :].reshape(c, KT, P).transpose(2, 1, 0)  # [p, j, t]
        outp[:, KT * m0:KT * (m0 + c)] = blk.reshape(P, KT * c)
        m0 += c
    return outp


def _pack_w(wg):
    """[512, 512] -> [128, KT*512]: w_shuf[p, j*512+n] = w[j*128+p, n]."""
    return np.ascontiguousarray(
        wg.reshape(KT, P, N).transpose(1, 0, 2).reshape(P, KT * N)
    ).astype(NPBF16)


def _unpack_out(o_shuf):
    """[128, NT*TOKENS] packed -> [TOKENS, 512]."""
    out = np.empty((TOKENS, N), dtype=np.float32)
    m0 = 0
    for c in CHUNKS:
        blk = o_shuf[:, NT * m0:NT * (m0 + c)].reshape(P, NT, c)
        out[m0:m0 + c, :] = blk.transpose(2, 1, 0).reshape(c, N)
        m0 += c
    return out


def _in_maps(x, blocks):
    return [
        {
            "xT": _pack_x(np.asarray(x[:, g * M:(g + 1) * M]).astype(NPBF16)),
            "w": _pack_w(np.asarray(blocks[g])),
        }
        for g in range(G)
    ]


def kernel(x, blocks):
    x = np.asarray(x)
    blocks = np.asarray(blocks)
    res = _run(_in_maps(x, blocks))
    return np.concatenate(
        [_unpack_out(res.results[g]["outT"].astype(np.float32)) for g in range(G)],
        axis=1,
    )


# revision 9
# speedup vs baseline: 1.0551x; 1.0551x over previous
"""Block-diagonal linear (grouped GEMM) on 8 TRN2 NeuronCores — v7.

out[b, g*512+n] = sum_k x[b, g*512+k] * blocks[g, k, n]

Group-parallel: core g computes block g's GEMM in bf16 (fp32 PSUM).

The host packs x / w / out in SBUF layout per token-chunk
(xT_shuf[p, KT*m0 + j*c + t] = x[m0+t, g*512+j*128+p]) so every
input/output transfer is a 2D DMA of 128 fat descriptors — each
in-flight DMA then carries 0.25-1MB, and ~30 DMAs cover the whole
kernel, far under the queue's in-flight cap.

Ring scheduling is explicit, because each HWDGE ring round-robins
packets across its outstanding transfers (~180GB/s per ring): a big
transfer issued next to a critical small one starves it. The plan
below gives each chunk a ring and a split so arrival tracks the
matmul stream's consumption with margin:

  sync:   W(j0), W(j123), c1(j01), c1(j23), c3, c5, c7, ...
  scalar: c0(j0), c0(j1), c0(j23), c2(j01), c2(j23), c4, c6, ...

plus per-chunk output DMAs on alternating rings. The warm-up dummy
matmuls cover the issue->first-data window (HAM un-throttles right as
real work starts); tail casts split across DVE and ACT; the final
32-token chunk packs 4 nt-groups in one PSUM bank whose flat layout
IS the packed output layout (one cast + one 32KB DMA at the end).
"""
import numpy as np
import ml_dtypes

import concourse.bacc as bacc
import concourse.tile as tile
from concourse import mybir
from concourse.bass_utils import run_bass_kernel_spmd

TOKENS = 8192
G = 8
M = 512
N = 512
P = 128
KT = M // P
NT = N // P
SUB = 512
F32 = mybir.dt.float32
BF16 = mybir.dt.bfloat16
NPBF16 = ml_dtypes.bfloat16

CHUNKS = [512, 512, 1024, 1024, 1024, 1024, 1024, 1024, 512, 256, 128, 96, 32]
assert sum(CHUNKS) == TOKENS
N_WARM = 20

# per-chunk input plan: (ring, tuple of j-split boundaries)
# ring: 0=sync 1=scalar; splits as j-ranges, e.g. ((0,1),(1,2),(2,4))
_IN_PLAN = {
    0: (1, ((0, 1), (1, 2), (2, 4))),
    1: (0, ((0, 2), (2, 4))),
    2: (1, ((0, 2), (2, 4))),
}


def _in_plan(ci):
    if ci in _IN_PLAN:
        return _IN_PLAN[ci]
    return (0 if ci % 2 == 1 else 1, ((0, 4),))


_CACHE: dict = {}


def _body(tc, nc, xT, w, outT):
    engs = (nc.sync, nc.scalar)
    with (
        tc.tile_pool(name="wp", bufs=1) as wp,
        tc.tile_pool(name="xin", bufs=8) as xin,
        tc.tile_pool(name="outp", bufs=3) as outp,
        tc.tile_pool(name="pso", bufs=8, space="PSUM") as pso,
    ):
        w_r = wp.tile([P, KT, N], BF16, tag="wr")

        # Head issues: W j0 + chunk0 j0 first on their rings.
        c0 = CHUNKS[0]
        x_t0 = xin.tile([P, KT * c0], BF16, tag="x", name="x_t0")
        nc.sync.dma_start(w_r[:, 0, :], w[:, 0:N])
        nc.scalar.dma_start(x_t0[:, :c0], xT[:, 0:c0])
        nc.sync.dma_start(w_r[:, 1:4, :], w[:, N:KT * N])
        nc.scalar.dma_start(x_t0[:, c0:2 * c0], xT[:, c0:2 * c0])
        nc.scalar.dma_start(x_t0[:, 2 * c0:KT * c0], xT[:, 2 * c0:KT * c0])

        # HAM warm-up: dependency-free dummy matmuls over zeroed SBUF
        # into a scratch PSUM bank while the first DMAs land.
        warm_x = xin.tile([P, 2 * P], BF16, tag="wx")
        warm_ps = pso.tile([P, SUB], F32, tag="pso")
        nc.vector.memset(warm_x, 0)
        for _ in range(N_WARM):
            nc.tensor.matmul(
                warm_ps[:, :P], warm_x[:, :P], warm_x[:, P:2 * P],
                start=True, stop=True,
            )
        # throwaway ACT copy: pulls the one-time ~1.3us ACT_TABLE_LOAD
        # into the warm-up shadow instead of the tail casts below
        nc.scalar.copy(warm_x[:, P:2 * P], warm_ps[:, :P])

        m0 = 0
        for ci, c in enumerate(CHUNKS):
            ring, splits = _in_plan(ci)
            if ci == 0:
                x_t = x_t0
            else:
                x_t = xin.tile([P, KT * c], BF16, tag="x")
                for (ja, jb) in splits:
                    engs[ring].dma_start(
                        x_t[:, ja * c:jb * c],
                        xT[:, KT * m0 + ja * c:KT * m0 + jb * c],
                    )
            xv = x_t.rearrange("p (j t) -> p j t", j=KT)
            oeng = engs[1 - ring]

            if ci == len(CHUNKS) - 1:
                # final 32-token chunk: 4 nt-groups in ONE PSUM bank;
                # the bank's flat [nt, t] layout IS the packed output
                # layout -> one cast + one small DMA
                ps_o = pso.tile([P, SUB], F32, tag="pso")
                for nt in range(NT):
                    for j in range(KT):
                        nc.tensor.matmul(
                            ps_o[:, nt * c:(nt + 1) * c],
                            w_r[:, j, nt * P:(nt + 1) * P],
                            xv[:, j, :],
                            start=(j == 0),
                            stop=(j == KT - 1),
                        )
                otl = outp.tile([P, NT * c], BF16, tag="o", name="otl")
                nc.vector.tensor_copy(otl, ps_o[:, :NT * c])
                oeng.dma_start(outT[:, NT * m0:NT * (m0 + c)], otl)
                m0 += c
                continue

            ot = outp.tile([P, NT * c], BF16, tag="o")
            ov = ot.rearrange("p (nt t) -> p nt t", nt=NT)
            if ci == 0:
                # j-outer: the first 4 matmuls need only w_j0 + x_j0
                pss0 = [
                    pso.tile([P, SUB], F32, tag="pso", name=f"ps0_{nt}")
                    for nt in range(NT)
                ]
                for j in range(KT):
                    for nt in range(NT):
                        nc.tensor.matmul(
                            pss0[nt][:, :c],
                            w_r[:, j, nt * P:(nt + 1) * P],
                            xv[:, j, :],
                            start=(j == 0),
                            stop=(j == KT - 1),
                        )
                for nt in range(NT):
                    nc.vector.tensor_copy(ov[:, nt, :], pss0[nt][:, :c])
            else:
                for s0 in range(0, c, SUB):
                    sw = min(SUB, c - s0)
                    for nt in range(NT):
                        ps_o = pso.tile([P, SUB], F32, tag="pso")
                        for j in range(KT):
                            nc.tensor.matmul(
                                ps_o[:, :sw],
                                w_r[:, j, nt * P:(nt + 1) * P],
                                xv[:, j, s0:s0 + sw],
                                start=(j == 0),
                                stop=(j == KT - 1),
                            )
                        # tail chunks: split casts over DVE and ACT so
                        # the drain after the last matmuls stays short
                        if ci >= len(CHUNKS) - 4 and nt % 2 == 1:
                            nc.scalar.copy(ov[:, nt, s0:s0 + sw], ps_o[:, :sw])
                        else:
                            nc.vector.tensor_copy(ov[:, nt, s0:s0 + sw], ps_o[:, :sw])
            oeng.dma_start(outT[:, NT * m0:NT * (m0 + c)], ot)
            m0 += c


def _build():
    nc = bacc.Bacc("TRN2", target_bir_lowering=False, debug=False, num_devices=G)
    xT = nc.dram_tensor("xT", [P, KT * TOKENS], BF16, kind="ExternalInput").ap()
    w = nc.dram_tensor("w", [P, KT * N], BF16, kind="ExternalInput").ap()
    outT = nc.dram_tensor("outT", [P, NT * TOKENS], BF16, kind="ExternalOutput").ap()
    with tile.TileContext(nc) as tc:
        _body(tc, nc, xT, w, outT)
    nc.compile()
    return nc


def _run(in_maps, **kwargs):
    if "nc" not in _CACHE:
        _CACHE["nc"] = _build()
    return run_bass_kernel_spmd(_CACHE["nc"], in_maps, list(range(G)), **kwargs)


def _pack_x(xg):
    """[TOKENS, 512] -> [128, KT*TOKENS] chunk-packed SBUF layout."""
    outp = np.empty((P, KT * TOKENS), dtype=NPBF16)
    m0 = 0
    for c in CHUNKS:
        blk = xg[m0:m0 + c, :].reshape(c, KT, P).transpose(2, 1, 0)  # [p, j, t]
        outp[:, KT * m0:KT * (m0 + c)] = blk.reshape(P, KT * c)
        m0 += c
    return outp


def _pack_w(wg):
    """[512, 512] -> [128, KT*512]: w_shuf[p, j*512+n] = w[j*128+p, n]."""
    return np.ascontiguousarray(
        wg.reshape(KT, P, N).transpose(1, 0, 2).reshape(P, KT * N)
    ).astype(NPBF16)


def _unpack_out(o_shuf):
    """[128, NT*TOKENS] packed -> [TOKENS, 512]."""
    out = np.empty((TOKENS, N), dtype=np.float32)
    m0 = 0
    for c in CHUNKS:
        blk = o_shuf[:, NT * m0:NT * (m0 + c)].reshape(P, NT, c)
        out[m0:m0 + c, :] = blk.transpose(2, 1, 0).reshape(c, N)
        m0 += c
    return out


def _in_maps(x, blocks):
    return [
        {
            "xT": _pack_x(np.asarray(x[:, g * M:(g + 1) * M]).astype(NPBF16)),
            "w": _pack_w(np.asarray(blocks[g])),
        }
        for g in range(G)
    ]


def kernel(x, blocks):
    x = np.asarray(x)
    blocks = np.asarray(blocks)
    res = _run(_in_maps(x, blocks))
    return np.concatenate(
        [_unpack_out(res.results[g]["outT"].astype(np.float32)) for g in range(G)],
        axis=1,
    )
